# revision 62
# baseline (speedup 1.0000x reference)
"""Multi-head attention (b=2, n=4096, h=8, d=64) on 8 trn2 cores.

Sharding: core c handles batch c//4, heads (2*(c%4), 2*(c%4)+1) — i.e. a
[4096, 128] embedding slice of q/k/v. Per core, a flash-style fused
attention runs fully on-chip:

  - Q, K slices are loaded, cast to bf16, and PE-transposed into [d, n]
    layout ("QT"/"KT", both heads packed into 128 partitions: rows 0-63 =
    head0, 64-127 = head1).
  - For each head / 512-wide q-chunk: S^T[kv_tile, q] = (KT tile).T @ QT
    lands in PSUM fp32; exp(S * scale) runs on the scalar engine straight
    out of PSUM into bf16 SBUF tiles (no max subtraction: |scores*scale|
    < ~6 for these inputs, far from fp32 overflow).
  - O^T accumulates in PSUM via matmul with V augmented by a ones column,
    which makes row 64 of the accumulator the softmax denominator.
  - Epilogue: scalar-engine copies out of PSUM, sum broadcast across
    partitions via a DRAM bounce, vector reciprocal+mul, DMA out as
    O^T [128, 4096] per core; host transposes/concats.

The target walrus build allows only ONE sync wait per instruction, so the
kernel is structured so every instruction has at most one cross-engine
dependency: epilogue/stage tiles are never slot-recycled (unique tensors),
V is staged before Q/K so the first matmul's DVE wait covers it, and
redundant same-engine waits are stripped post-scheduling.
"""

import sys

sys.path.insert(0, "/opt/trn_rl_repo")

import numpy as np

import concourse.bass as bass
import concourse.tile as tile
from concourse import mybir
from concourse.bass_utils import run_bass_kernel_spmd
from concourse.masks import make_identity
from concourse.tile_rust import add_dep_helper

B, N, E = 2, 4096, 512
D = 64  # head dim
P = 128
NT = N // P  # 32 kv/row tiles
QC = 512  # q-chunk (one PSUM bank of fp32)
NQC = N // QC  # 8
SCALE = D ** -0.5
F32 = mybir.dt.float32
BF16 = mybir.dt.bfloat16
EXP = mybir.ActivationFunctionType.Exp

# kv-tile groups per exp ACTIVATE: 3 tiles -> [128, 1536] fp32 = 3 PSUM banks
GW = 3
GROUPS = [(j, min(GW, NT - j)) for j in range(0, NT, GW)]
CH = 8  # t-tiles per DMA chunk


def _build() -> bass.Bass:
    nc = bass.Bass()
    q = nc.dram_tensor("q", [N, P], F32, kind="ExternalInput")
    k = nc.dram_tensor("k", [N, P], F32, kind="ExternalInput")
    v = nc.dram_tensor("v", [N, P], F32, kind="ExternalInput")
    ot = nc.dram_tensor("ot", [P, N], F32, kind="ExternalOutput")

    with tile.TileContext(nc) as tc:
        with (
            tc.tile_pool(name="persist", bufs=1) as pp,
            tc.tile_pool(name="load", bufs=1) as lp,
            tc.tile_pool(name="ptpool", bufs=2) as ptp,
            tc.tile_pool(name="epi", bufs=1) as ep,
            tc.tile_pool(name="dscratch", bufs=1, space="DRAM") as dsp,
            tc.tile_pool(name="opsum", bufs=2, space="PSUM") as op_,
        ):
            tpsum_cm = tc.tile_pool(name="tpsum", bufs=2, space="PSUM")
            tpp = tpsum_cm.__enter__()
            # score-tile pool A: 2 kv-tiles per exp while transposes still
            # hold 2 PSUM banks (2*2 + 2 + po 2 = 8 banks)
            spa_cm = tc.tile_pool(name="spsumA", bufs=2, space="PSUM")
            spa = spa_cm.__enter__()

            # identity for PE transpose, bridged through DVE; a dummy
            # transpose absorbs the ident dependency so real transposes
            # carry only their data (DVE) wait
            identg = pp.tile([P, P], BF16)
            make_identity(nc, identg)
            ident = pp.tile([P, P], BF16)
            nc.vector.tensor_copy(ident, identg)
            scr = tpp.tile([P, P], BF16, tag="tp")
            nc.tensor.transpose(scr, ident, ident)
            last = {}  # last instruction per proc, for the tail nop chain
            poolscr = pp.tile([1, 1], F32)
            last["pool"] = nc.gpsimd.memset(poolscr, 0.0)
            dmas = []  # DMA instructions in program order

            qt = pp.tile([P, N], BF16)  # rows 0-63: head0 Q^T; 64-127: head1
            kt = pp.tile([P, N], BF16)
            va0 = pp.tile([P, NT, D + 1], BF16)  # V tiles + ones column, head0
            va1 = pp.tile([P, NT, D + 1], BF16)

            # ones columns first: tiny DVE ticks, trivially covered by every
            # later matmul's DVE wait
            nc.vector.memset(va0[:, :, D:D + 1], 1.0)
            nc.vector.memset(va1[:, :, D:D + 1], 1.0)
            onesrow = pp.tile([1, D], BF16)
            nc.vector.memset(onesrow, 1.0)

            v_r = v.rearrange("(t p) e -> p t e", p=P)
            q_r = q.rearrange("(t p) e -> p t e", p=P)
            k_r = k.rearrange("(t p) e -> p t e", p=P)
            vst = lp.tile([P, NT, P], F32, tag="stage_v")
            qst = lp.tile([P, NT, P], F32, tag="stage_q")
            kst = lp.tile([P, NT, P], F32, tag="stage_k")
            qb = lp.tile([P, NT, P], BF16, tag="cast_q")
            kb = lp.tile([P, NT, P], BF16, tag="cast_k")

            gdmas = []  # gpsimd (SWDGE) loads
            tpcopies = []  # PSUM->SBUF transpose copies (DVE)
            epi_dve = []  # epilogue DVE readers of PSUM accumulators

            def load_t(src_r, xst, xb, dstT, t0, eng, ch=CH, cp=None):
                tsl = slice(t0, t0 + ch)
                d = eng.dma_start(out=xst[:, tsl, :], in_=src_r[:, tsl, :])
                (gdmas if eng is nc.gpsimd else dmas).append(d)
                copyf = nc.scalar.copy if cp == "act" else nc.vector.tensor_copy
                copyf(xb[:, tsl, :], xst[:, tsl, :])
                for t in range(t0, t0 + ch):
                    ptt = tpp.tile([P, P], BF16, tag="tp")
                    nc.tensor.transpose(ptt, xb[:, t, :], ident)
                    tpcopies.append(copyf(dstT[:, t * P:(t + 1) * P], ptt))

            chunk_po = {}
            po_readers = {}
            chunk_idx = [0]

            op_hold = [op_]

            def open_chunk(h, qc, ngroups):
                chunk_po[(h, qc)] = op_hold[0].tile(
                    [D + 1, QC], F32, tag="po", name=f"po_{h}_{qc}"
                )
                chunk_groups_left[(h, qc)] = ngroups

            pv_pending = []  # (h, qc, j0, glen, va, ptg) awaiting PV matmuls
            chunk_groups_left = {}

            def emit_pv_one():
                if not pv_pending:
                    return
                h, qc, j0, glen, va, ptg = pv_pending.pop(0)
                po = chunk_po[(h, qc)]
                for jj in range(glen):
                    j = j0 + jj
                    last["pe"] = nc.tensor.matmul(
                        po,
                        va[:, j, :],
                        ptg[:, jj * QC:(jj + 1) * QC],
                        start=(j == 0),
                        stop=(j == NT - 1),
                    )
                chunk_groups_left[(h, qc)] -= 1
                if chunk_groups_left[(h, qc)] == 0:
                    close_chunk(h, qc)

            def emit_group(h, qc, j0, glen, va, spool, gw):
                # scores + exp now; PV deferred one group so the PE stream
                # always has the next scores ahead of the blocked PV
                hb = h * D
                qs = slice(qc * QC, (qc + 1) * QC)
                sg = spool.tile(
                    [P, gw * QC], F32, tag=f"sg{gw}", name=f"sg_{h}_{qc}_{j0}"
                )
                for jj in range(glen):
                    j = j0 + jj
                    nc.tensor.matmul(
                        sg[:, jj * QC:(jj + 1) * QC],
                        kt[hb:hb + D, j * P:(j + 1) * P],
                        qt[hb:hb + D, qs],
                        start=True,
                        stop=True,
                    )
                ptg = ptp.tile([P, GW * QC], BF16, tag="pt", name=f"pt_{h}_{qc}_{j0}")
                last["act"] = nc.scalar.activation(
                    ptg[:, :glen * QC], sg[:, :glen * QC], EXP, scale=SCALE
                )
                pv_pending.append((h, qc, j0, glen, va, ptg))
                while len(pv_pending) > 1:
                    emit_pv_one()

            def close_chunk(h, qc):
                hb = h * D
                qs = slice(qc * QC, (qc + 1) * QC)
                po = chunk_po.pop((h, qc))
                u = f"{h}_{qc}"
                # DVE-only po readers: the slot's reuse costs a single wait
                # (parked on a ldweights by the post-pass)
                rcp = ep.tile([1, QC], BF16, tag=f"rcp_{u}")
                with nc.allow_low_precision("softmax denominator in bf16"):
                    epi_dve.append(nc.vector.reciprocal(rcp, po[D:D + 1, :]))
                rbc = ep.tile([D, QC], BF16, tag=f"rbc_{u}")
                if chunk_idx[0] == 15:
                    # final chunk: broadcast via PE (ones @ rcp) + ACT copy —
                    # no DMA 900ns sem-prop latencies on the critical tail
                    bc_ps = spb.tile([D, QC], F32, tag="sg3", name=f"bc_{u}")
                    last["pe"] = nc.tensor.matmul(
                        bc_ps, onesrow, rcp, start=True, stop=True
                    )
                    last["act"] = nc.scalar.copy(rbc, bc_ps)
                else:
                    rcp_d = dsp.tile([1, QC], BF16, tag=f"rcpd_{u}")
                    dmas.append(nc.sync.dma_start(out=rcp_d, in_=rcp))
                    dmas.append(
                        nc.sync.dma_start(out=rbc, in_=rcp_d.broadcast_to([D, QC]))
                    )
                osb = ep.tile([D, QC], F32, tag=f"osb_{u}")
                mul = nc.vector.tensor_mul(osb, po[0:D, :], rbc)
                epi_dve.append(mul)
                last["dve"] = mul
                po_readers[chunk_idx[0]] = mul
                chunk_idx[0] += 1
                dmas.append(nc.sync.dma_start(out=ot[hb:hb + D, qs], in_=osb))

            # Load rounds (q, v, k per round; k on the gpsimd SWDGE
            # stream) with chunk (0,0)/(0,1) groups woven in, so PE overlaps
            # transposes with score matmuls in its in-order stream and ACT
            # never starves. Round 0 is split so the first scores land
            # early. V copies stay below the K transpose-copies in DVE tick
            # order, keeping the O^T matmuls' va dependency covered by the
            # S^T matmuls' DVE wait.
            open_chunk(0, 0, 16)
            rounds = [  # (t0, ch, [(chunk_qc, [groups...]), ...])
                (0, 4, [(0, [0, 1])]),
                (4, 4, [(0, [2, 3]), (1, [0, 1, 2, 3])]),
                (8, 8, [(0, [4, 5, 6, 7]), (1, [4, 5, 6, 7])]),
                (16, 8, [(0, [8, 9, 10, 11]), (1, [8, 9, 10, 11])]),
                (24, 8, [(0, [12, 13, 14, 15]), (1, [12, 13, 14, 15])]),
            ]
            for t0, ch, emits in rounds:
                tsl = slice(t0, t0 + ch)
                load_t(q_r, qst, qb, qt, t0, nc.sync, ch)
                dmas.append(nc.sync.dma_start(out=vst[:, tsl, :], in_=v_r[:, tsl, :]))
                if t0 > 0:
                    for h, va in ((0, va0), (1, va1)):
                        nc.vector.tensor_copy(
                            va[:, tsl, 0:D], vst[:, tsl, h * D:(h + 1) * D]
                        )
                load_t(k_r, kst, kb, kt, t0, nc.gpsimd, ch)
                if t0 == 0:
                    # round 0: K path first on DVE; the first PV's va wait is
                    # hoisted onto its ldweights by the post-pass
                    for h, va in ((0, va0), (1, va1)):
                        nc.vector.tensor_copy(
                            va[:, tsl, 0:D], vst[:, tsl, h * D:(h + 1) * D]
                        )
                for qc, gs in emits:
                    if (0, qc) not in chunk_po:
                        open_chunk(0, qc, 16)
                    for g in gs:
                        emit_group(0, qc, 2 * g, 2, va0, spa, 2)
            while pv_pending:
                emit_pv_one()

            # ACT bridge: observe the transpose-copies (DVE) so the first
            # wider-pool exp's released-zone dep costs no extra wait; kept
            # alive via the tail chain
            actbridge = ep.tile([1, 1], BF16)
            _bridge_bi = nc.scalar.copy(actbridge, ident[0:1, 0:1])
            for tpc in tpcopies:
                add_dep_helper(
                    _bridge_bi.ins, tpc.ins, sync=True, reason="act-dve-bridge"
                )

            # transpose + A-score banks recycle into the wider B-score pool
            spa_cm.__exit__(None, None, None)
            tpsum_cm.__exit__(None, None, None)
            spb_cm = tc.tile_pool(name="spsumB", bufs=2, space="PSUM")
            spb = spb_cm.__enter__()

            first_steady = [True]
            for h, va in ((0, va0), (1, va1)):
                for qc in range(NQC):
                    if (h, qc) in ((0, 0), (0, 1)):
                        continue
                    open_chunk(h, qc, len(GROUPS))
                    for j0, glen in GROUPS:
                        emit_group(h, qc, j0, glen, va, spb, GW)
                        if first_steady[0]:
                            # keep the ACT bridge scheduled before the first
                            # steady exp (ordering only, no semaphore)
                            add_dep_helper(
                                last["act"].ins, _bridge_bi.ins, sync=False,
                                reason="bridge-order",
                            )
                            first_steady[0] = False
            while pv_pending:
                emit_pv_one()

            spb_cm.__exit__(None, None, None)

            # tail chain: one sync-engine nop per outstanding proc so the
            # framework's kernel-tail drain needs no multi-proc waits
            for dep in [last["pool"], last["pe"], last["act"], last["dve"], _bridge_bi] + gdmas + dmas[-16:] + dmas[:8]:
                nop = nc.sync.nop()
                add_dep_helper(nop.ins, dep.ins, sync=True, reason="tail-chain")
    return nc


# Engines execute their instruction streams in order, so a semaphore wait on
# an engine's own semaphore is redundant with FIFO issue order (walrus also
# caps sync waits at 1 for most opcodes, which such waits overflow).
# Pool (gpsimd) is deliberately absent: its 8 DSP cores can overlap
# consecutive instructions, so its self-waits are load-bearing.
_SELF_SEM = {
    "PE": "PE_",
    "Activation": "Activation_",
    "DVE": "DVE_",
}
_KEEP_OPCODES = {"Drain", "EventSemaphore"}


def _strip_redundant_waits(nc: bass.Bass) -> None:
    """Drop waits that engine/queue FIFO order already guarantees:

    - an engine instruction waiting on its own engine's semaphore
      (engines execute and complete in order);
    - a DMA instruction waiting on the semaphore of its own assigned
      DMA lane (one lane = one HW queue; transfers on a queue are FIFO).
    """
    for bb in nc.main_func.blocks:
        for ins in bb.instructions:
            if ins.opcode in _KEEP_OPCODES:
                continue
            si = ins.sync_info
            if si is None or not si.on_wait:
                continue
            prefixes = []
            eng = str(getattr(ins, "engine", "")).split(".")[-1]
            if eng in _SELF_SEM:
                prefixes.append(_SELF_SEM[eng])
            upd = si.on_update or []
            for u in upd:
                name = getattr(u, "ant_name", None)
                if name and (name.startswith("DMAHW") or name.startswith("DMASW")):
                    prefixes.append(name.rsplit("_", 1)[0] + "_")
            if not prefixes:
                continue
            kept = [
                w
                for w in si.on_wait
                if not (
                    w.sync_type == "semaphore"
                    and w.ant_name is not None
                    and w.wait_mode == "sem-ge-imm"
                    and any(w.ant_name.startswith(p) for p in prefixes)
                )
            ]
            if len(kept) != len(si.on_wait):
                ins.sync_info = mybir.SyncInfo(on_wait=kept, on_update=si.on_update)


def _hoist_matmult_waits(nc: bass.Bass) -> None:
    """walrus allows one sync wait per Matmult; when a matmult carries two
    (accumulator-slot reuse + rhs producer), park the extra wait on the
    Ldweights that immediately precedes it on the PE stream (it executes
    first, so waiting there is strictly conservative)."""
    for bb in nc.main_func.blocks:
        prev = None
        for ins in bb.instructions:
            if (
                ins.opcode == "Matmult"
                and prev is not None
                and prev.opcode == "Ldweights"
            ):
                si = ins.sync_info
                psi = prev.sync_info
                prev_waits = list(psi.on_wait) if psi and psi.on_wait else []
                if si and si.on_wait and len(si.on_wait) > 1 and not prev_waits:
                    waits = list(si.on_wait)
                    moved = waits[:-1]
                    keep = waits[-1:]
                    prev.sync_info = mybir.SyncInfo(
                        on_wait=moved,
                        on_update=list(psi.on_update) if psi else [],
                    )
                    ins.sync_info = mybir.SyncInfo(
                        on_wait=keep, on_update=si.on_update
                    )
            prev = ins


_NC = None


def _get_nc() -> bass.Bass:
    global _NC
    if _NC is None:
        _NC = _build()
        _strip_redundant_waits(_NC)
        _hoist_matmult_waits(_NC)
    return _NC


def kernel(q: np.ndarray, k: np.ndarray, v: np.ndarray) -> np.ndarray:
    q = np.asarray(q, dtype=np.float32)
    k = np.asarray(k, dtype=np.float32)
    v = np.asarray(v, dtype=np.float32)
    in_maps = []
    for c in range(8):
        b, hp = c // 4, c % 4
        sl = slice(hp * P, (hp + 1) * P)
        in_maps.append(
            {
                "q": np.ascontiguousarray(q[b, :, sl]),
                "k": np.ascontiguousarray(k[b, :, sl]),
                "v": np.ascontiguousarray(v[b, :, sl]),
            }
        )
    results = run_bass_kernel_spmd(_get_nc(), in_maps, list(range(8))).results
    out = np.empty((B, N, E), dtype=np.float32)
    for c in range(8):
        b, hp = c // 4, c % 4
        out[b, :, hp * P:(hp + 1) * P] = results[c]["ot"].T
    return out
